# revision 17
# baseline (speedup 1.0000x reference)
"""Multi-head attention (B=2, S=2048, D=1024, H=16) on 8 trn2 NeuronCores.

Sharding: data-parallel over (batch, query-chunk): core c handles batch c//4,
query rows [512*(c%4), 512*(c%4)+512). Each core projects Q for its rows and
K/V for the full batch (duplicated within the 4-core batch group), runs
attention for its rows, and writes its output slice. No collectives.

All device layouts are feature-major so no on-chip transposes are needed:
  - inputs passed as query.T/key.T/value.T [D, rows], weights as W.T [in, out]
  - projections produce Q.T/K.T [out_feat, rows] and V [rows, out_feat]
  - scores computed transposed [k, q]; softmax across k (partitions):
      exp on ACT with mask folded into the per-partition bias,
      denominator via a ones-row appended to V in the attn@V matmul,
      normalization via a K=1 broadcast matmul + one DVE multiply
  - 1/sqrt(dk) folded into wq host-side; bv/bo folded into bo' = bo + wo@bv
"""

import sys

for _p in ("/opt/trn_rl_repo", "/root/.axon_site/_ro/trn_rl_repo"):
    if _p not in sys.path:
        sys.path.insert(0, _p)

import numpy as np
import ml_dtypes

B, S, D, H, DK = 2, 2048, 1024, 16, 64
NCORES = 8
MQ = 512          # query rows per core
P = 128           # partitions
NQT = MQ // P     # 4 query row-tiles
NIT = D // P      # 8 input-feature tiles
NOT_ = D // P     # 8 output-feature tiles
NKT = S // P      # 16 key tiles
NKC = S // 512    # 4 key/row chunks of 512
VW = DK + 1       # 65: head dim + ones row

BF16 = ml_dtypes.bfloat16

_CACHE = {}


def _build(loop_n=None):
    from concourse import bacc
    import concourse.mybir as mybir
    import concourse.tile as tile

    nc = bacc.Bacc("TRN2", target_bir_lowering=False, debug=False)
    dt = mybir.dt

    qT = nc.dram_tensor("qT", [D, MQ], dt.bfloat16, kind="ExternalInput")
    kT = nc.dram_tensor("kT", [D, S], dt.bfloat16, kind="ExternalInput")
    vT = nc.dram_tensor("vT", [D, S], dt.bfloat16, kind="ExternalInput")
    wq = nc.dram_tensor("wq", [D, D], dt.bfloat16, kind="ExternalInput")
    wk = nc.dram_tensor("wk", [D, D], dt.bfloat16, kind="ExternalInput")
    wv = nc.dram_tensor("wv", [D, D], dt.bfloat16, kind="ExternalInput")
    wo = nc.dram_tensor("wo", [D, D], dt.bfloat16, kind="ExternalInput")
    bq = nc.dram_tensor("bq", [P, NOT_], dt.float32, kind="ExternalInput")
    bk = nc.dram_tensor("bk", [P, NOT_], dt.float32, kind="ExternalInput")
    maskb = nc.dram_tensor("maskb", [P, NKT], dt.float32, kind="ExternalInput")
    bob = nc.dram_tensor("bob", [1, D], dt.float32, kind="ExternalInput")
    onesr = nc.dram_tensor("onesr", [1, DK], dt.float32r, kind="ExternalInput")
    out = nc.dram_tensor("out", [MQ, D], dt.float32, kind="ExternalOutput")

    import bass_rust  # noqa: F401
    import concourse.bass as bass

    with tile.TileContext(nc) as tc:
        with (
            tc.tile_pool(name="w", bufs=2) as wpool,
            tc.tile_pool(name="stat", bufs=1) as stat,
            tc.tile_pool(name="inT", bufs=2) as inpool,
            tc.tile_pool(name="qin", bufs=1) as qin,
            tc.tile_pool(name="big", bufs=1) as big,
            tc.tile_pool(name="pT", bufs=6) as ppool,
            tc.tile_pool(name="sm", bufs=4) as sm,
            tc.tile_pool(name="outp", bufs=2) as outp,
            tc.tile_pool(name="psA", bufs=4, space="PSUM") as psA,
            tc.tile_pool(name="psB", bufs=2, space="PSUM") as psB,
        ):
            _body_ctx = (
                tc.For_i(
                    0,
                    loop_n,
                    1,
                    hint_engines=(
                        mybir.EngineType.PE,
                        mybir.EngineType.Activation,
                        mybir.EngineType.DVE,
                        mybir.EngineType.SP,
                    ),
                )
                if loop_n is not None
                else None
            )
            if _body_ctx is not None:
                _body_ctx.__enter__()

            # ---- persistent tiles ----
            QT_sb = big.tile([P, NOT_, MQ], dt.bfloat16, tag="QT")
            KT_sb = big.tile([P, NOT_, S], dt.bfloat16, tag="KT")
            Vp_sb = big.tile([P, NKT, H * VW], dt.bfloat16, tag="Vp")
            ctx_sb = big.tile([P, NOT_, MQ], dt.bfloat16, tag="ctx")
            bq_sb = stat.tile([P, NOT_], dt.float32, tag="bq")
            bk_sb = stat.tile([P, NOT_], dt.float32, tag="bk")
            mb_sb = stat.tile([P, NKT], dt.float32, tag="mb")
            bob_sb = stat.tile([P, D], dt.float32, tag="bob")
            ones_sb = stat.tile([1, DK], dt.float32r, tag="ones")

            nc.sync.dma_start(out=bq_sb, in_=bq[:, :])
            nc.sync.dma_start(out=bk_sb, in_=bk[:, :])
            nc.sync.dma_start(out=mb_sb, in_=maskb[:, :])
            bob_bcast = bass.AP(
                tensor=bob.ap().tensor, offset=0, ap=[[0, P], [1, D]]
            )
            nc.sync.dma_start(out=bob_sb, in_=bob_bcast)
            nc.sync.dma_start(out=ones_sb, in_=onesr[:, :])
            # ones rows of V' (column DK of each head block)
            vones = Vp_sb.rearrange("p t (h x) -> p t h x", x=VW)[:, :, :, DK : DK + 1]
            nc.vector.memset(vones, 1.0)

            def load_w(name, dram):
                t = wpool.tile([P, NIT, D], dt.bfloat16, tag="w", name=name)
                nc.sync.dma_start(
                    out=t, in_=dram.ap().rearrange("(t p) o -> p t o", p=P)
                )
                return t

            # ---- Q projection: Q.T[o, q] ----
            wq_sb = load_w("wq_sb", wq)
            qT_sb = qin.tile([P, NIT, MQ], dt.bfloat16, tag="qTin")
            nc.sync.dma_start(out=qT_sb, in_=qT.ap().rearrange("(t p) q -> p t q", p=P))
            for ot in range(NOT_):
                ps = psA.tile([P, 512], dt.float32, tag="ps1")
                for it in range(NIT):
                    nc.tensor.matmul(
                        ps,
                        lhsT=wq_sb[:, it, ot * P : (ot + 1) * P],
                        rhs=qT_sb[:, it, :],
                        start=(it == 0),
                        stop=(it == NIT - 1),
                    )
                nc.vector.tensor_scalar_add(
                    out=QT_sb[:, ot, :], in0=ps, scalar1=bq_sb[:, ot : ot + 1]
                )

            # ---- K projection: K.T[o, k] ----
            wk_sb = load_w("wk_sb", wk)
            for kc in range(NKC):
                kTc = inpool.tile([P, NIT, 512], dt.bfloat16, tag="inT")
                nc.sync.dma_start(
                    out=kTc,
                    in_=kT[:, kc * 512 : (kc + 1) * 512].rearrange(
                        "(t p) k -> p t k", p=P
                    ),
                )
                for ot in range(NOT_):
                    ps = psA.tile([P, 512], dt.float32, tag="ps1")
                    for it in range(NIT):
                        nc.tensor.matmul(
                            ps,
                            lhsT=wk_sb[:, it, ot * P : (ot + 1) * P],
                            rhs=kTc[:, it, :],
                            start=(it == 0),
                            stop=(it == NIT - 1),
                        )
                    nc.vector.tensor_scalar_add(
                        out=KT_sb[:, ot, kc * 512 : (kc + 1) * 512],
                        in0=ps,
                        scalar1=bk_sb[:, ot : ot + 1],
                    )

            # ---- V projection: V[k_row, o], interleaved with ones rows ----
            wv_sb = load_w("wv_sb", wv)
            for rc in range(NKC):
                vTc = inpool.tile([P, NIT, 512], dt.bfloat16, tag="inT")
                nc.sync.dma_start(
                    out=vTc,
                    in_=vT[:, rc * 512 : (rc + 1) * 512].rearrange(
                        "(t p) k -> p t k", p=P
                    ),
                )
                for rt in range(4):
                    kt = rc * 4 + rt
                    for oc in range(2):
                        ps = psA.tile([P, 512], dt.float32, tag="ps1")
                        for it in range(NIT):
                            nc.tensor.matmul(
                                ps,
                                lhsT=vTc[:, it, rt * P : (rt + 1) * P],
                                rhs=wv_sb[:, it, oc * 512 : (oc + 1) * 512],
                                start=(it == 0),
                                stop=(it == NIT - 1),
                            )
                        dst = Vp_sb[
                            :, kt, oc * 8 * VW : (oc * 8 + 8) * VW
                        ].rearrange("p (h x) -> p h x", x=VW)[:, :, 0:DK]
                        nc.vector.tensor_copy(
                            out=dst, in_=ps.rearrange("p (h x) -> p h x", x=DK)
                        )

            # ---- attention, head pairs (2j, 2j+1), kt-interleaved ----
            for j in range(H // 2):
                ps_av = [
                    psA.tile([P, 512], dt.float32, tag="ps1", name=f"ps_av_{j}_{m}")
                    for m in range(2)
                ]
                for kt in range(NKT):
                    sc = psB.tile([P, 1024], dt.float32, tag="ps2")
                    nc.tensor.matmul(
                        sc[:, 0:512],
                        lhsT=KT_sb[0:DK, j, kt * P : (kt + 1) * P],
                        rhs=QT_sb[0:DK, j, :],
                        start=True,
                        stop=True,
                        tile_position=(0, 0),
                    )
                    nc.tensor.matmul(
                        sc[:, 512:1024],
                        lhsT=KT_sb[DK:P, j, kt * P : (kt + 1) * P],
                        rhs=QT_sb[DK:P, j, :],
                        start=True,
                        stop=True,
                        tile_position=(DK, 0),
                    )
                    p_kt = ppool.tile([P, 1024], dt.bfloat16, tag="pT")
                    nc.scalar.activation(
                        out=p_kt,
                        in_=sc,
                        func=mybir.ActivationFunctionType.Exp,
                        bias=mb_sb[:, kt : kt + 1],
                        scale=1.0,
                    )
                    for hh in range(2):
                        nc.tensor.matmul(
                            ps_av[hh][0:VW, :],
                            lhsT=Vp_sb[:, kt, (2 * j + hh) * VW : (2 * j + hh + 1) * VW],
                            rhs=p_kt[:, hh * 512 : (hh + 1) * 512],
                            start=(kt == 0),
                            stop=(kt == NKT - 1),
                            skip_group_check=True,
                        )
                for hh in range(2):
                    h = 2 * j + hh
                    recip = sm.tile([1, 512], dt.float32r, tag="recip")
                    with nc.allow_low_precision(reason="fp32r keeps 19 mantissa bits"):
                        nc.vector.reciprocal(
                            out=recip, in_=ps_av[hh][DK : DK + 1, :]
                        )
                    ps_bc = psA.tile([P, 512], dt.float32, tag="ps1")
                    nc.tensor.matmul(
                        ps_bc[0:DK, :],
                        lhsT=ones_sb,
                        rhs=recip,
                        start=True,
                        stop=True,
                    )
                    av_sb = sm.tile([DK, 512], dt.float32, tag="avsb")
                    nc.vector.tensor_copy(out=av_sb, in_=ps_av[hh][0:DK, :])
                    nc.vector.tensor_mul(
                        out=ctx_sb[hh * DK : (hh + 1) * DK, h // 2, :],
                        in0=av_sb,
                        in1=ps_bc[0:DK, :],
                    )

            # ---- output projection ----
            wo_sb = load_w("wo_sb", wo)
            for qt in range(NQT):
                for oc in range(2):
                    ps = psA.tile([P, 512], dt.float32, tag="ps1")
                    for jt in range(NIT):
                        nc.tensor.matmul(
                            ps,
                            lhsT=ctx_sb[:, jt, qt * P : (qt + 1) * P],
                            rhs=wo_sb[:, jt, oc * 512 : (oc + 1) * 512],
                            start=(jt == 0),
                            stop=(jt == NIT - 1),
                        )
                    o_sb = outp.tile([P, 512], dt.float32, tag="osb")
                    nc.vector.tensor_add(
                        out=o_sb, in0=ps, in1=bob_sb[:, oc * 512 : (oc + 1) * 512]
                    )
                    nc.sync.dma_start(
                        out=out[qt * P : (qt + 1) * P, oc * 512 : (oc + 1) * 512],
                        in_=o_sb,
                    )

            if _body_ctx is not None:
                _body_ctx.__exit__(None, None, None)

    nc.finalize()
    return nc


def _get_nc():
    if "nc" not in _CACHE:
        _CACHE["nc"] = _build()
    return _CACHE["nc"]


def _make_inputs(query, key, value, mask, wq, bq, wk, bk, wv, bv, wo, bo):
    query = np.asarray(query, dtype=np.float32)
    key = np.asarray(key, dtype=np.float32)
    value = np.asarray(value, dtype=np.float32)
    mask = np.asarray(mask)
    f32 = np.float32
    wqT = np.ascontiguousarray(np.asarray(wq, f32).T / 8.0).astype(BF16)
    wkT = np.ascontiguousarray(np.asarray(wk, f32).T).astype(BF16)
    wvT = np.ascontiguousarray(np.asarray(wv, f32).T).astype(BF16)
    woT = np.ascontiguousarray(np.asarray(wo, f32).T).astype(BF16)
    bq8 = np.ascontiguousarray((np.asarray(bq, f32) / 8.0).reshape(NOT_, P).T)
    bkr = np.ascontiguousarray(np.asarray(bk, f32).reshape(NOT_, P).T)
    bob = (np.asarray(bo, f32) + np.asarray(wo, f32) @ np.asarray(bv, f32))[None, :]
    bob = np.ascontiguousarray(bob)
    onesr = np.ones((1, DK), dtype=f32)

    in_maps = []
    for c in range(NCORES):
        b = c // 4
        q0 = (c % 4) * MQ
        qTc = np.ascontiguousarray(query[b].T[:, q0 : q0 + MQ]).astype(BF16)
        kTc = np.ascontiguousarray(key[b].T).astype(BF16)
        vTc = np.ascontiguousarray(value[b].T).astype(BF16)
        mbias = np.where(mask[b, 0, 0] == 0, f32(-1e5), f32(0.0)).astype(f32)
        mbias = np.ascontiguousarray(mbias.reshape(NKT, P).T)
        in_maps.append(
            {
                "qT": qTc,
                "kT": kTc,
                "vT": vTc,
                "wq": wqT,
                "wk": wkT,
                "wv": wvT,
                "wo": woT,
                "bq": bq8,
                "bk": bkr,
                "maskb": mbias,
                "bob": bob,
                "onesr": onesr,
            }
        )
    return in_maps


def kernel(query, key, value, mask, wq, bq, wk, bk, wv, bv, wo, bo):
    from concourse.bass_utils import run_bass_kernel_spmd

    nc = _get_nc()
    in_maps = _make_inputs(
        query, key, value, mask, wq, bq, wk, bk, wv, bv, wo, bo
    )
    res = run_bass_kernel_spmd(nc, in_maps, core_ids=list(range(NCORES)))
    out = np.empty((B, S, D), dtype=np.float32)
    for c in range(NCORES):
        b = c // 4
        q0 = (c % 4) * MQ
        out[b, q0 : q0 + MQ, :] = res.results[c]["out"]
    return out
